# revision 37
# baseline (speedup 1.0000x reference)
"""ContinuousMask kernel for Trainium2 (8 NeuronCores, SPMD row-sharded).

Problem: starts[B=2048, N=8192] int32, T=16384, l=1638. Output bool [B, T]:
True everywhere except the union of windows [s, s+l) over each row's starts.

Algorithm (per row):
  A position t is covered iff some start lies in (t-l, t]. With value-chunks
  of width W=512 (2W <= l), if every chunk 0..28 contains at least one start,
  then the covered region is EXACTLY [smin, smax+l), smin < 512, and
  smax+l >= 15974, so the mask is fully described by a head strip [0, 512)
  (True iff t < smin) and a tail strip [TSTART, T) (True iff t >= smax+l);
  the constant-False middle is never stored (run_bass_kernel_spmd's PJRT
  path donates zero-initialized output buffers). Chunk occupancy is checked
  on a WITNESS SUBSET of columns (passing PROVES the condition; failing only
  flags the row for exact host recompute — on the target distribution a
  512-column witness flags a row with P ~ 5e-7).

Engine split (per 128-row tile), measured on TRN2 hardware:
  - loads: 4 quarter-loads interleaved across both HWDGE queues (SP + Act),
    which together sustain ~385 GB/s/core — the binding roofline (8 MB of
    starts per core; the body time equals the pure-load floor).
  - DVE does ALL compute: min and max via custom two-stream reduce DVE ops
    (registered per-NEFF below; the stock InstTensorTensorReduce wedges this
    hardware and Pool/Act have no usable integer ops), chained over quarter
    pairs through the accumulator-init operand; witness shift/shift-left/
    or-tree for occupancy; both strip paints (GPSIMD paints measured 5x
    slower; Act's activation pays an act-table load per instruction).
  - stores ride the SWDGE (gpsimd) queue so the HWDGE queues stay pure loads.
  - the or-tree stops at 32 columns; the final OR + flag test runs on host
    (DVE tensor_reduce has no bitwise_or).
"""

import numpy as np

B = 2048
T = 16384
NSEG = 8192
L = 1638
NCORES = 8
RPC = B // NCORES  # 256 rows per core
PT = 128  # rows per partition tile
NRT = RPC // PT  # 2 row tiles per core
Q = NSEG // 4  # quarter width (2048)
SHIFT = 9  # occupancy chunk width 512 (2*512 <= L)
OCC_COLS = 512  # occupancy witness column count
# Require witness occupancy of ALL chunks 0..28 (values span [0, 14747), so
# chunk 28 is the last). Chunk 0 occupied => smin < 512; chunk 28 occupied =>
# smax >= 14336 => the tail True-run starts at smax+L >= 15974.
MIN_CLAST = 29
HSTRIP = 512  # head strip [0, 512) covers [0, smin) since smin < 512
TSTART = T - 512  # tail strip [15872, T) covers runs starting >= 15974

_prog_cache: dict = {}


def _register_minmax_ops():
    """Register two-stream min/max reduce custom DVE ops (per-NEFF uop table;
    the documented extension path — no firmware change). The stock
    InstTensorTensorReduce wedges this hardware, so these replace it:
    one instruction streams two operand ranges (2 elems/cycle/lane) and
    folds the pairwise result into the fp32 accumulator."""
    import concourse.dve_ops as dve_ops
    from concourse.dve_ops import DveOp
    from concourse.dve_spec import Spec, Src0, Src1, C0, minn, maxx, lower
    from concourse.dve_spec import _has_src1
    from concourse.dve_uop import DveOpSpec

    existing = {op.name: op for op in dve_ops.OPS}
    if "TT_MIN_REDUCE_X" in existing:
        return existing["TT_MIN_REDUCE_X"], existing["TT_MAX_REDUCE_X"]

    def _ref(np_op):
        fold = np.min if np_op is np.minimum else np.max

        def ref(in0, in1, c0, c1, c2):
            out = np_op(
                np.asarray(in0).astype(np.float32), np.asarray(in1).astype(np.float32)
            )
            acc = np_op(np.asarray(c0, np.float32), fold(out, axis=-1, keepdims=True))
            return out, acc

        return ref

    def make(name, body, accum, np_op):
        spec = Spec(body=body, accum=accum, accum_init=C0, reference=_ref(np_op))
        row = 1 + len(dve_ops.OPS)
        assert row < 0x20, "custom DVE row overflow"
        dve_ops._SUB_OPCODE_FOR_NAME[name] = row
        uops = lower(spec, ver="v3")
        sha = DveOpSpec(
            name=name, opcode=row, uops=uops, rd1_en=_has_src1(spec)
        ).sha("v3")
        op = DveOp(name=name, spec=spec, subdim=False, uops_sha={"v3": sha})
        dve_ops.OPS.append(op)
        dve_ops.CUSTOM_DVE_SPECS[name] = spec
        return op

    mn = make("TT_MIN_REDUCE_X", minn(Src0, Src1), minn, np.minimum)
    mx = make("TT_MAX_REDUCE_X", maxx(Src0, Src1), maxx, np.maximum)
    return mn, mx


def _build_program(reps: int = 1, mode: str = "full", nbufs: int = 2, halves: bool = False, rows_split: bool = False, eighths: int = 0):
    """mode: 'full' | 'dma'/'dma3'/'load' (no compute) | 'compute[:part]'."""
    import concourse.bacc as bacc
    import concourse.mybir as mybir
    from concourse.tile import TileContext

    MN_OP, MX_OP = _register_minmax_ops()

    dt = mybir.dt
    Alu = mybir.AluOpType
    X = mybir.AxisListType.X

    nc = bacc.Bacc("TRN2", debug=False)
    starts_d = nc.declare_dram_parameter("starts", [RPC, NSEG], dt.int32, isOutput=False)
    mask_d = nc.declare_dram_parameter("mask", [RPC, T], dt.uint8, isOutput=True)
    flags_d = nc.declare_dram_parameter("flags", [RPC, 32], dt.int32, isOutput=True)

    with TileContext(nc) as tc:
        with (
            tc.tile_pool(name="persist", bufs=1) as pp,
            tc.tile_pool(name="stp", bufs=nbufs) as stp,
            tc.tile_pool(name="scratch", bufs=2) as scp,
            tc.tile_pool(name="strip", bufs=4) as outp,
            tc.tile_pool(name="small", bufs=4) as sp,
        ):
            iota_t = pp.tile([PT, HSTRIP], dt.int16, tag="iota")
            nc.gpsimd.iota(iota_t[:], [[1, HSTRIP]], base=0, channel_multiplier=0)
            ones_t = pp.tile([PT, OCC_COLS], dt.int32, tag="ones")
            nc.vector.memset(ones_t[:], 1)

            persist_st: dict = {}
            for rep in range(reps):
              for rt in range(NRT):
                r0 = rt * PT
                is_compute = mode.startswith("compute")
                parts = mode.split(":")[1] if ":" in mode else "all"
                do_load = not is_compute or rep == 0
                do_compute = not mode.startswith("dma") and mode != "load"
                do_store = not is_compute and mode != "load"

                if is_compute:
                    if rt not in persist_st:
                        st_persist = pp.tile([PT, NSEG], dt.int32, tag=f"st{rt}")
                        persist_st[rt] = st_persist
                    st = persist_st[rt]
                else:
                    st = stp.tile([PT, NSEG], dt.int32, tag="st")
                if do_load:
                    if mode == "dma3":
                        # thirds across sync/scalar HWDGE + gpsimd SWDGE
                        TH = NSEG // 4
                        nc.sync.dma_start(out=st[:, 0:TH], in_=starts_d[r0 : r0 + PT, 0:TH])
                        nc.scalar.dma_start(out=st[:, TH : 2 * TH], in_=starts_d[r0 : r0 + PT, TH : 2 * TH])
                        nc.gpsimd.dma_start(out=st[:, 2 * TH : 3 * TH], in_=starts_d[r0 : r0 + PT, 2 * TH : 3 * TH])
                        nc.sync.dma_start(out=st[:, 3 * TH : 3 * TH + TH // 2], in_=starts_d[r0 : r0 + PT, 3 * TH : 3 * TH + TH // 2])
                        nc.scalar.dma_start(out=st[:, 3 * TH + TH // 2 : NSEG], in_=starts_d[r0 : r0 + PT, 3 * TH + TH // 2 : NSEG])
                    elif halves:
                        # one DMA per queue per tile (fewer HWDGE setups)
                        nc.sync.dma_start(out=st[:, 0 : 2 * Q], in_=starts_d[r0 : r0 + PT, 0 : 2 * Q])
                        nc.scalar.dma_start(out=st[:, 2 * Q : NSEG], in_=starts_d[r0 : r0 + PT, 2 * Q : NSEG])
                    elif rows_split:
                        # row-contiguous: each queue streams 64 whole rows
                        HP = PT // 2
                        nc.sync.dma_start(out=st[0:HP, :], in_=starts_d[r0 : r0 + HP, :])
                        nc.scalar.dma_start(out=st[HP:PT, :], in_=starts_d[r0 + HP : r0 + PT, :])
                    elif eighths:
                        E = NSEG // eighths  # eighths = number of column splits
                        for i in range(eighths):
                            eng = nc.sync if i % 2 == 0 else nc.scalar
                            eng.dma_start(
                                out=st[:, i * E : (i + 1) * E],
                                in_=starts_d[r0 : r0 + PT, i * E : (i + 1) * E],
                            )
                    else:
                        # quarter-loads, interleaved across both HWDGE queues so
                        # the first reduce pair can start at the half-way point
                        nc.sync.dma_start(out=st[:, 0:Q], in_=starts_d[r0 : r0 + PT, 0:Q])
                        nc.scalar.dma_start(out=st[:, Q : 2 * Q], in_=starts_d[r0 : r0 + PT, Q : 2 * Q])
                        nc.sync.dma_start(out=st[:, 2 * Q : 3 * Q], in_=starts_d[r0 : r0 + PT, 2 * Q : 3 * Q])
                        nc.scalar.dma_start(out=st[:, 3 * Q : NSEG], in_=starts_d[r0 : r0 + PT, 3 * Q : NSEG])
                if not do_compute:
                    if do_store:
                        ph0 = outp.tile([PT, HSTRIP], dt.uint8, tag="ph")
                        nc.vector.memset(ph0[:], 0)
                        nc.scalar.dma_start(out=mask_d[r0 : r0 + PT, 0:HSTRIP], in_=ph0[:])
                        pt0 = outp.tile([PT, T - TSTART], dt.uint8, tag="pt")
                        nc.vector.memset(pt0[:], 0)
                        nc.scalar.dma_start(out=mask_d[r0 : r0 + PT, TSTART:T], in_=pt0[:])
                    continue

                # fp32 accumulators: the DVE reduce accumulator is fp32; all
                # values here are < 2^20 so fp32 is exact
                smin = sp.tile([PT, 1], dt.float32, tag="smin")
                smax = sp.tile([PT, 1], dt.float32, tag="smax")
                mn0 = sp.tile([PT, 1], dt.float32, tag="mn0")
                mx0 = sp.tile([PT, 1], dt.float32, tag="mx0")
                dmy = sp.tile([PT, 1], dt.float32, tag="dmy")

                if parts in ("all", "mm"):
                    # min/max over quarter pairs (q0,q1) then (q2,q3), chaining
                    # through the accumulator initial value; the elementwise
                    # output is discarded via a stride-0 broadcast dummy
                    nc.vector._custom_dve(
                        MN_OP, out=dmy.broadcast_to((PT, Q)),
                        in0=st[:, 0:Q], in1=st[:, Q : 2 * Q],
                        s0=float(1 << 20), accum_out=mn0[:],
                    )
                    nc.vector._custom_dve(
                        MX_OP, out=dmy.broadcast_to((PT, Q)),
                        in0=st[:, 0:Q], in1=st[:, Q : 2 * Q],
                        s0=0.0, accum_out=mx0[:],
                    )
                    nc.vector._custom_dve(
                        MN_OP, out=dmy.broadcast_to((PT, Q)),
                        in0=st[:, 2 * Q : 3 * Q], in1=st[:, 3 * Q : NSEG],
                        s0=mn0[:], accum_out=smin[:],
                    )
                    nc.vector._custom_dve(
                        MX_OP, out=dmy.broadcast_to((PT, Q)),
                        in0=st[:, 2 * Q : 3 * Q], in1=st[:, 3 * Q : NSEG],
                        s0=mx0[:], accum_out=smax[:],
                    )
                else:
                    nc.vector.memset(smin[:], 3.0)
                    nc.vector.memset(smax[:], 14500.0)

                if parts in ("all", "occ"):
                    # witness occupancy: shift (needs only q0), bit set, or-tree
                    # down to 32 columns; final OR + flag test on host
                    hi = scp.tile([PT, OCC_COLS], dt.int32, tag="hi")
                    nc.vector.tensor_scalar(hi[:], st[:, 0:OCC_COLS], SHIFT, None, Alu.arith_shift_right)
                    bits = scp.tile([PT, OCC_COLS], dt.int32, tag="bits")
                    nc.vector.tensor_tensor(bits[:], ones_t[:], hi[:], Alu.logical_shift_left)
                    w = OCC_COLS
                    while w > 32:
                        h = w // 2
                        nc.vector.tensor_tensor(
                            bits[:, 0:h], bits[:, 0:h], bits[:, h:w], Alu.bitwise_or
                        )
                        w = h
                    if do_store:
                        # stores ride the SWDGE (gpsimd) queue so both HWDGE
                        # queues stay pure input loads
                        nc.gpsimd.dma_start(out=flags_d[r0 : r0 + PT, :], in_=bits[:, 0:32])

                if parts in ("all", "paint"):
                    # paint strips, all on DVE: head (t < smin), tail
                    # (t >= smax + L - TSTART); the tail threshold is a tiny
                    # per-partition add
                    smaxl_f = sp.tile([PT, 1], dt.float32, tag="smaxlf")
                    nc.vector.tensor_scalar(
                        smaxl_f[:], smax[:], float(L - TSTART), None, Alu.add
                    )
                    ph = outp.tile([PT, HSTRIP], dt.uint8, tag="ph")
                    pt = outp.tile([PT, T - TSTART], dt.uint8, tag="pt")
                    nc.vector.tensor_scalar(ph[:], iota_t[:], smin[:], None, Alu.is_lt)
                    nc.vector.tensor_scalar(pt[:], iota_t[:], smaxl_f[:], None, Alu.is_ge)
                    if do_store:
                        nc.gpsimd.dma_start(out=mask_d[r0 : r0 + PT, 0:HSTRIP], in_=ph[:])
                        nc.gpsimd.dma_start(out=mask_d[r0 : r0 + PT, TSTART:T], in_=pt[:])

    nc.finalize()
    return nc


def _get_program(reps: int = 1, mode: str = "full", nbufs: int = 2, halves: bool = False, rows_split: bool = False, eighths: int = 0):
    key = (reps, mode, nbufs, halves, rows_split, eighths)
    if key not in _prog_cache:
        _prog_cache[key] = _build_program(
            reps, mode, nbufs=nbufs, halves=halves, rows_split=rows_split,
            eighths=eighths,
        )
    return _prog_cache[key]


def _host_exact_row(row_starts: np.ndarray) -> np.ndarray:
    delta = np.zeros(T + 1, np.int64)
    np.add.at(delta, row_starts, 1)
    np.add.at(delta, row_starts + L, -1)
    return ~(np.cumsum(delta)[:T] > 0)


def run_device(starts: np.ndarray, trace: bool = False):
    """Run the SPMD bass kernel. Returns (mask_u8 [B,T], flags [B], results)."""
    from concourse.bass_utils import run_bass_kernel_spmd

    nc = _get_program()
    shards = starts.reshape(NCORES, RPC, NSEG)
    in_maps = [{"starts": np.ascontiguousarray(shards[c])} for c in range(NCORES)]
    res = run_bass_kernel_spmd(nc, in_maps, list(range(NCORES)), trace=trace)
    mask = np.concatenate([r["mask"] for r in res.results], axis=0)
    occ32 = np.concatenate([r["flags"] for r in res.results], axis=0)  # [B, 32]
    occ = np.bitwise_or.reduce(occ32.astype(np.int64), axis=1)
    flags = ((occ | (-1 << MIN_CLAST)) != -1).astype(np.int32)
    return mask, flags, res


def kernel(**inputs) -> np.ndarray:
    starts = np.ascontiguousarray(np.asarray(inputs["starts"]), dtype=np.int32)
    t_in = int(np.asarray(inputs["T"]))
    l_in = int(np.asarray(inputs["l"]))
    assert starts.shape == (B, NSEG), starts.shape
    assert t_in == T and l_in == L, (t_in, l_in)

    mask_u8, flags, _ = run_device(starts)
    mask = mask_u8.astype(bool)

    bad_rows = np.nonzero(flags != 0)[0]
    for r in bad_rows:  # pathological rows: exact host recompute (rare)
        mask[r] = _host_exact_row(starts[r])
    return mask


# revision 41
# speedup vs baseline: 1.1330x; 1.1330x over previous
"""ContinuousMask kernel for Trainium2 (8 NeuronCores, SPMD row-sharded).

Problem: starts[B=2048, N=8192] int32, T=16384, l=1638. Output bool [B, T]:
True everywhere except the union of windows [s, s+l) over each row's starts.

Algorithm (per row):
  A position t is covered iff some start lies in (t-l, t]. With value-chunks
  of width W=512 (2W <= l), if every chunk 0..28 contains at least one start,
  then the covered region is EXACTLY [smin, smax+l), smin < 512, and
  smax+l >= 15974, so the mask is fully described by a head strip [0, 512)
  (True iff t < smin) and a tail strip [TSTART, T) (True iff t >= smax+l);
  the constant-False middle is never stored (run_bass_kernel_spmd's PJRT
  path donates zero-initialized output buffers). Chunk occupancy is checked
  on a WITNESS SUBSET of columns (passing PROVES the condition; failing only
  flags the row for exact host recompute — on the target distribution a
  512-column witness flags a row with P ~ 5e-7).

Engine split (per 128-row tile), measured on TRN2 hardware:
  - loads: 4 quarter-loads interleaved across both HWDGE queues (SP + Act),
    which together sustain ~385 GB/s/core — the binding roofline (8 MB of
    starts per core; the body time equals the pure-load floor).
  - DVE does ALL compute: min and max via custom two-stream reduce DVE ops
    (registered per-NEFF below; the stock InstTensorTensorReduce wedges this
    hardware and Pool/Act have no usable integer ops), chained over quarter
    pairs through the accumulator-init operand; witness shift/shift-left/
    or-tree for occupancy; both strip paints (GPSIMD paints measured 5x
    slower; Act's activation pays an act-table load per instruction).
  - stores ride the SWDGE (gpsimd) queue so the HWDGE queues stay pure loads.
  - the or-tree stops at 32 columns; the final OR + flag test runs on host
    (DVE tensor_reduce has no bitwise_or).
"""

import numpy as np

B = 2048
T = 16384
NSEG = 8192
L = 1638
NCORES = 8
RPC = B // NCORES  # 256 rows per core
PT = 128  # rows per partition tile
NRT = RPC // PT  # 2 row tiles per core
Q = NSEG // 4  # quarter width (2048)
SHIFT = 9  # occupancy chunk width 512 (2*512 <= L)
OCC_COLS = 512  # occupancy witness column count
# Require witness occupancy of ALL chunks 0..28 (values span [0, 14747), so
# chunk 28 is the last). Chunk 0 occupied => smin < 512; chunk 28 occupied =>
# smax >= 14336 => the tail True-run starts at smax+L >= 15974.
MIN_CLAST = 29
HSTRIP = 512  # head strip [0, 512) covers [0, smin) since smin < 512
TSTART = T - 512  # tail strip [15872, T) covers runs starting >= 15974

_prog_cache: dict = {}


def _register_minmax_ops():
    """Register two-stream min/max reduce custom DVE ops (per-NEFF uop table;
    the documented extension path — no firmware change). The stock
    InstTensorTensorReduce wedges this hardware, so these replace it:
    one instruction streams two operand ranges (2 elems/cycle/lane) and
    folds the pairwise result into the fp32 accumulator."""
    import concourse.dve_ops as dve_ops
    from concourse.dve_ops import DveOp
    from concourse.dve_spec import Spec, Src0, Src1, C0, minn, maxx, lower
    from concourse.dve_spec import _has_src1
    from concourse.dve_uop import DveOpSpec

    existing = {op.name: op for op in dve_ops.OPS}
    if "TT_MIN_REDUCE_X" in existing:
        return existing["TT_MIN_REDUCE_X"], existing["TT_MAX_REDUCE_X"]

    def _ref(np_op):
        fold = np.min if np_op is np.minimum else np.max

        def ref(in0, in1, c0, c1, c2):
            out = np_op(
                np.asarray(in0).astype(np.float32), np.asarray(in1).astype(np.float32)
            )
            acc = np_op(np.asarray(c0, np.float32), fold(out, axis=-1, keepdims=True))
            return out, acc

        return ref

    def make(name, body, accum, np_op):
        spec = Spec(body=body, accum=accum, accum_init=C0, reference=_ref(np_op))
        row = 1 + len(dve_ops.OPS)
        assert row < 0x20, "custom DVE row overflow"
        dve_ops._SUB_OPCODE_FOR_NAME[name] = row
        uops = lower(spec, ver="v3")
        sha = DveOpSpec(
            name=name, opcode=row, uops=uops, rd1_en=_has_src1(spec)
        ).sha("v3")
        op = DveOp(name=name, spec=spec, subdim=False, uops_sha={"v3": sha})
        dve_ops.OPS.append(op)
        dve_ops.CUSTOM_DVE_SPECS[name] = spec
        return op

    mn = make("TT_MIN_REDUCE_X", minn(Src0, Src1), minn, np.minimum)
    mx = make("TT_MAX_REDUCE_X", maxx(Src0, Src1), maxx, np.maximum)
    return mn, mx


def _build_program(reps: int = 1, mode: str = "full", nbufs: int = 2, halves: bool = False, rows_split: bool = False, eighths: int = 0):
    """mode: 'full' | 'dma'/'dma3'/'load' (no compute) | 'compute[:part]'."""
    import concourse.bacc as bacc
    import concourse.mybir as mybir
    from concourse.tile import TileContext

    MN_OP, MX_OP = _register_minmax_ops()

    dt = mybir.dt
    Alu = mybir.AluOpType
    X = mybir.AxisListType.X

    nc = bacc.Bacc("TRN2", debug=False)
    starts_d = nc.declare_dram_parameter("starts", [RPC, NSEG], dt.int16, isOutput=False)
    mask_d = nc.declare_dram_parameter("mask", [RPC, T], dt.uint8, isOutput=True)
    flags_d = nc.declare_dram_parameter("flags", [RPC, 32], dt.int32, isOutput=True)

    with TileContext(nc) as tc:
        with (
            tc.tile_pool(name="persist", bufs=1) as pp,
            tc.tile_pool(name="stp", bufs=nbufs) as stp,
            tc.tile_pool(name="scratch", bufs=2) as scp,
            tc.tile_pool(name="strip", bufs=4) as outp,
            tc.tile_pool(name="small", bufs=4) as sp,
        ):
            iota_t = pp.tile([PT, HSTRIP], dt.int16, tag="iota")
            nc.gpsimd.iota(iota_t[:], [[1, HSTRIP]], base=0, channel_multiplier=0)
            ones_t = pp.tile([PT, OCC_COLS], dt.int32, tag="ones")
            nc.vector.memset(ones_t[:], 1)

            persist_st: dict = {}
            for rep in range(reps):
              for rt in range(NRT):
                r0 = rt * PT
                is_compute = mode.startswith("compute")
                parts = mode.split(":")[1] if ":" in mode else "all"
                do_load = not is_compute or rep == 0
                do_compute = not mode.startswith("dma") and mode != "load"
                do_store = not is_compute and mode != "load"

                if is_compute:
                    if rt not in persist_st:
                        st_persist = pp.tile([PT, NSEG], dt.int16, tag=f"st{rt}")
                        persist_st[rt] = st_persist
                    st = persist_st[rt]
                else:
                    st = stp.tile([PT, NSEG], dt.int16, tag="st")
                if do_load:
                    if mode == "dma3":
                        # thirds across sync/scalar HWDGE + gpsimd SWDGE
                        TH = NSEG // 4
                        nc.sync.dma_start(out=st[:, 0:TH], in_=starts_d[r0 : r0 + PT, 0:TH])
                        nc.scalar.dma_start(out=st[:, TH : 2 * TH], in_=starts_d[r0 : r0 + PT, TH : 2 * TH])
                        nc.gpsimd.dma_start(out=st[:, 2 * TH : 3 * TH], in_=starts_d[r0 : r0 + PT, 2 * TH : 3 * TH])
                        nc.sync.dma_start(out=st[:, 3 * TH : 3 * TH + TH // 2], in_=starts_d[r0 : r0 + PT, 3 * TH : 3 * TH + TH // 2])
                        nc.scalar.dma_start(out=st[:, 3 * TH + TH // 2 : NSEG], in_=starts_d[r0 : r0 + PT, 3 * TH + TH // 2 : NSEG])
                    elif halves:
                        # one DMA per queue per tile (fewer HWDGE setups)
                        nc.sync.dma_start(out=st[:, 0 : 2 * Q], in_=starts_d[r0 : r0 + PT, 0 : 2 * Q])
                        nc.scalar.dma_start(out=st[:, 2 * Q : NSEG], in_=starts_d[r0 : r0 + PT, 2 * Q : NSEG])
                    elif rows_split:
                        # row-contiguous: each queue streams 64 whole rows
                        HP = PT // 2
                        nc.sync.dma_start(out=st[0:HP, :], in_=starts_d[r0 : r0 + HP, :])
                        nc.scalar.dma_start(out=st[HP:PT, :], in_=starts_d[r0 + HP : r0 + PT, :])
                    elif eighths:
                        E = NSEG // eighths  # eighths = number of column splits
                        for i in range(eighths):
                            eng = nc.sync if i % 2 == 0 else nc.scalar
                            eng.dma_start(
                                out=st[:, i * E : (i + 1) * E],
                                in_=starts_d[r0 : r0 + PT, i * E : (i + 1) * E],
                            )
                    else:
                        # quarter-loads, interleaved across both HWDGE queues so
                        # the first reduce pair can start at the half-way point
                        nc.sync.dma_start(out=st[:, 0:Q], in_=starts_d[r0 : r0 + PT, 0:Q])
                        nc.scalar.dma_start(out=st[:, Q : 2 * Q], in_=starts_d[r0 : r0 + PT, Q : 2 * Q])
                        nc.sync.dma_start(out=st[:, 2 * Q : 3 * Q], in_=starts_d[r0 : r0 + PT, 2 * Q : 3 * Q])
                        nc.scalar.dma_start(out=st[:, 3 * Q : NSEG], in_=starts_d[r0 : r0 + PT, 3 * Q : NSEG])
                if not do_compute:
                    if do_store:
                        ph0 = outp.tile([PT, HSTRIP], dt.uint8, tag="ph")
                        nc.vector.memset(ph0[:], 0)
                        nc.scalar.dma_start(out=mask_d[r0 : r0 + PT, 0:HSTRIP], in_=ph0[:])
                        pt0 = outp.tile([PT, T - TSTART], dt.uint8, tag="pt")
                        nc.vector.memset(pt0[:], 0)
                        nc.scalar.dma_start(out=mask_d[r0 : r0 + PT, TSTART:T], in_=pt0[:])
                    continue

                # fp32 accumulators: the DVE reduce accumulator is fp32; all
                # values here are < 2^20 so fp32 is exact
                smin = sp.tile([PT, 1], dt.float32, tag="smin")
                smax = sp.tile([PT, 1], dt.float32, tag="smax")
                mn0 = sp.tile([PT, 1], dt.float32, tag="mn0")
                mx0 = sp.tile([PT, 1], dt.float32, tag="mx0")
                dmy = sp.tile([PT, 1], dt.int16, tag="dmy")

                if parts in ("all", "mm"):
                    # min/max over quarter pairs (q0,q1) then (q2,q3), chaining
                    # through the accumulator initial value; the elementwise
                    # output is discarded via a stride-0 broadcast dummy
                    nc.vector._custom_dve(
                        MN_OP, out=dmy.broadcast_to((PT, Q)),
                        in0=st[:, 0:Q], in1=st[:, Q : 2 * Q],
                        s0=float(1 << 20), accum_out=mn0[:],
                    )
                    nc.vector._custom_dve(
                        MX_OP, out=dmy.broadcast_to((PT, Q)),
                        in0=st[:, 0:Q], in1=st[:, Q : 2 * Q],
                        s0=0.0, accum_out=mx0[:],
                    )
                    nc.vector._custom_dve(
                        MN_OP, out=dmy.broadcast_to((PT, Q)),
                        in0=st[:, 2 * Q : 3 * Q], in1=st[:, 3 * Q : NSEG],
                        s0=mn0[:], accum_out=smin[:],
                    )
                    nc.vector._custom_dve(
                        MX_OP, out=dmy.broadcast_to((PT, Q)),
                        in0=st[:, 2 * Q : 3 * Q], in1=st[:, 3 * Q : NSEG],
                        s0=mx0[:], accum_out=smax[:],
                    )
                else:
                    nc.vector.memset(smin[:], 3.0)
                    nc.vector.memset(smax[:], 14500.0)

                if parts in ("all", "occ"):
                    # witness occupancy: shift (needs only q0), bit set, or-tree
                    # down to 32 columns; final OR + flag test on host.
                    # shifts only exist for int32, and bitvec ops cannot cast,
                    # so widen the witness columns first (arith add casts).
                    wit32 = scp.tile([PT, OCC_COLS], dt.int32, tag="wit32")
                    nc.vector.tensor_scalar(wit32[:], st[:, 0:OCC_COLS], 0.0, None, Alu.add)
                    hi = scp.tile([PT, OCC_COLS], dt.int32, tag="hi")
                    nc.vector.tensor_scalar(hi[:], wit32[:], SHIFT, None, Alu.arith_shift_right)
                    bits = scp.tile([PT, OCC_COLS], dt.int32, tag="bits")
                    nc.vector.tensor_tensor(bits[:], ones_t[:], hi[:], Alu.logical_shift_left)
                    w = OCC_COLS
                    while w > 32:
                        h = w // 2
                        nc.vector.tensor_tensor(
                            bits[:, 0:h], bits[:, 0:h], bits[:, h:w], Alu.bitwise_or
                        )
                        w = h
                    if do_store:
                        # stores ride the SWDGE (gpsimd) queue so both HWDGE
                        # queues stay pure input loads
                        nc.gpsimd.dma_start(out=flags_d[r0 : r0 + PT, :], in_=bits[:, 0:32])

                if parts in ("all", "paint"):
                    # paint strips, all on DVE: head (t < smin), tail
                    # (t >= smax + L - TSTART); the tail threshold is a tiny
                    # per-partition add
                    smaxl_f = sp.tile([PT, 1], dt.float32, tag="smaxlf")
                    nc.vector.tensor_scalar(
                        smaxl_f[:], smax[:], float(L - TSTART), None, Alu.add
                    )
                    ph = outp.tile([PT, HSTRIP], dt.uint8, tag="ph")
                    pt = outp.tile([PT, T - TSTART], dt.uint8, tag="pt")
                    nc.vector.tensor_scalar(ph[:], iota_t[:], smin[:], None, Alu.is_lt)
                    nc.vector.tensor_scalar(pt[:], iota_t[:], smaxl_f[:], None, Alu.is_ge)
                    if do_store:
                        nc.gpsimd.dma_start(out=mask_d[r0 : r0 + PT, 0:HSTRIP], in_=ph[:])
                        nc.gpsimd.dma_start(out=mask_d[r0 : r0 + PT, TSTART:T], in_=pt[:])

    nc.finalize()
    return nc


def _get_program(reps: int = 1, mode: str = "full", nbufs: int = 2, halves: bool = False, rows_split: bool = False, eighths: int = 0):
    key = (reps, mode, nbufs, halves, rows_split, eighths)
    if key not in _prog_cache:
        _prog_cache[key] = _build_program(
            reps, mode, nbufs=nbufs, halves=halves, rows_split=rows_split,
            eighths=eighths,
        )
    return _prog_cache[key]


def _host_exact_row(row_starts: np.ndarray) -> np.ndarray:
    delta = np.zeros(T + 1, np.int64)
    np.add.at(delta, row_starts, 1)
    np.add.at(delta, row_starts + L, -1)
    return ~(np.cumsum(delta)[:T] > 0)


def preprocess(starts: np.ndarray) -> np.ndarray:
    """Pack starts to int16 on host (values < 2^15, lossless) — halves the
    device's HBM read traffic."""
    assert starts.max() < (1 << 15) and starts.min() >= 0
    return np.ascontiguousarray(starts.astype(np.int16))


def run_device(starts: np.ndarray, trace: bool = False):
    """Run the SPMD bass kernel. Returns (mask_u8 [B,T], flags [B], results)."""
    from concourse.bass_utils import run_bass_kernel_spmd

    nc = _get_program()
    if starts.dtype != np.int16:
        starts = preprocess(starts)
    shards = starts.reshape(NCORES, RPC, NSEG)
    in_maps = [{"starts": np.ascontiguousarray(shards[c])} for c in range(NCORES)]
    res = run_bass_kernel_spmd(nc, in_maps, list(range(NCORES)), trace=trace)
    mask = np.concatenate([r["mask"] for r in res.results], axis=0)
    occ32 = np.concatenate([r["flags"] for r in res.results], axis=0)  # [B, 32]
    occ = np.bitwise_or.reduce(occ32.astype(np.int64), axis=1)
    flags = ((occ | (-1 << MIN_CLAST)) != -1).astype(np.int32)
    return mask, flags, res


def kernel(**inputs) -> np.ndarray:
    starts = np.ascontiguousarray(np.asarray(inputs["starts"]), dtype=np.int32)
    t_in = int(np.asarray(inputs["T"]))
    l_in = int(np.asarray(inputs["l"]))
    assert starts.shape == (B, NSEG), starts.shape
    assert t_in == T and l_in == L, (t_in, l_in)

    mask_u8, flags, _ = run_device(starts)
    mask = mask_u8.astype(bool)

    bad_rows = np.nonzero(flags != 0)[0]
    for r in bad_rows:  # pathological rows: exact host recompute (rare)
        mask[r] = _host_exact_row(starts[r])
    return mask
